# revision 20
# baseline (speedup 1.0000x reference)
"""Trainium2 Bass kernel for nn_AttentionModule_16398185136487.

Math (the reference reduces to this — its trailing softmax is over a size-1
axis, i.e. ones):
  out = concat([x34, a_x4, x43, b_x3], axis=1)            # (8, 512, 32, 32)
  block(qs, ks, v) = gate(qs, ks) * (w128@wv @ x_v + w128@bv) + b128
  gate(qs, ks)[b, hw] = softmax_hw( (1/8) sum_{kb} max_{khw}
                                    (Q_qs[b,hw] . K_ks[kb,khw]) / 16 )

Sharding: core j owns batch image j (its 1024 query pixels for both the x4
and x3 streams) — the per-image softmax is then fully core-local; no
collectives.  The K tensors (all 16 key images) are computed replicated on
every core from the full x4/x3 (a small 1x1 conv vs the score GEMM shard).

Engine plan per core (v2):
  - Q/K convs in bf16 with outputs quantized to fp8e4; the 1024 score
    matmuls per (q-tile, key-image) pair collapse to ONE fp8 DoubleRow
    matmul per 512-key half (K=256 contraction via the 2 stacked ci
    chunks, 2 MACs/cycle) — the score GEMM runs at 2x the bf16 rate and
    with half the LDWEIGHTS.
  - per-image max over 1024 keys is split across the two PSUM-capable
    elementwise engines (GPSIMD cannot touch PSUM; DMA cannot read it):
      method A (ScalarE): one-pass exp(BETA*(s-C)) with accum_out -> the
        per-row exp-sum; log once per A-column later (LSE ~ max for
        BETA=12; the -BETA*C bias keeps exp in fp32 range and its
        constant offset cancels in the per-image softmax because the
        A/C method split is uniform across each gate's 8 q-tiles).
      method C (VectorE): exact reduce_max over the 1024-col 2-bank
        PSUM tile in one op.
    Methods are assigned per (key-image, query-half) in a checkerboard
    so both engines run every iteration; qs order is interleaved
    [0,8,1,9,...] to alternate the halves in time.
  - V path in fp32r (full-rate fp32 matmul) with the two convs fused
    host-side (w128@wv).
  - per-image softmax without max-subtraction (logits are O(1)), gates
    broadcast to 128 partitions via K=1 PE matmuls, and a custom DVE op
    (GMUL_BIAS) applies out = gate_row * V * (1/S) + b128 in one pass.
"""

import numpy as np
import ml_dtypes

B = 8
C = 256
HW = 1024          # 32*32
BHW = B * HW       # 8192
NCORES = 8

BETA = 12.0        # LSE sharpness in raw-score units
CBIAS = 9.0        # exp bias: exp(BETA*(s - CBIAS)) stays in fp32 range

_CACHE = {}


def _ref_gmul_bias(in0, in1, c0, c1, c2):
    return (in0.astype(np.float32) * in1 * c1 + c0).astype(np.float32)


def _get_custom_ops():
    """Register the GMUL_BIAS custom DVE microcode op:
      GMUL_BIAS: out = in0 * in1 * s1 + s0     (s0, s1 per-partition APs)
    """
    if "ops" in _CACHE:
        return _CACHE["ops"]
    import concourse.dve_ops as dve_ops
    from concourse.dve_ops import DveOp
    from concourse.dve_spec import Spec, Src0, Src1, C0, C1, lower
    from concourse.dve_uop import DveOpSpec

    def register(name, spec):
        for op in dve_ops.OPS:
            if op.name == name:
                return op
        shas = {}
        for ver in ("v3", "v4"):
            shas[ver] = DveOpSpec(name=name, opcode=1,
                                  uops=lower(spec, ver=ver),
                                  rd1_en=True).sha(ver)
        op = DveOp(name, spec, subdim=False, uops_sha=shas)
        dve_ops.OPS.append(op)
        dve_ops.CUSTOM_DVE_SPECS[op.name] = op.spec
        dve_ops._SUB_OPCODE_FOR_NAME[op.name] = (
            dve_ops._CUSTOM_DVE_ROW_BASE + len(dve_ops.OPS) - 1)
        assert max(dve_ops._SUB_OPCODE_FOR_NAME.values()) < 0x20
        return op

    gmul = register("GMUL_BIAS",
                    Spec(body=Src0 * Src1 * C1 + C0,
                         reference=_ref_gmul_bias))
    _CACHE["ops"] = (gmul,)
    return _CACHE["ops"]


# method('A'=ScalarE LSE | 'C'=DVE exact max) per (img, qhalf); must be
# uniform across the 8 q-tiles of each gate so the -BETA*CBIAS offset of
# A-images cancels in that gate's softmax.  Image-PARITY assignment makes
# every (q-tile, image-pair) iteration feed BOTH engines (ia->A, ib->C or
# swapped), so ScalarE and VectorE run concurrently instead of in
# alternating bursts.
def _method(img, qh):
    return 'A' if (img + qh) % 2 == 0 else 'C'


def _build_nc():
    from contextlib import ExitStack

    import concourse.bass as bass
    import concourse.mybir as mybir
    import concourse.tile as tile
    from concourse import bacc
    from concourse.masks import make_identity

    f32 = mybir.dt.float32
    f32r = mybir.dt.float32r
    bf16 = mybir.dt.bfloat16
    fp8 = mybir.dt.float8e4
    AX = mybir.AxisListType.X
    AXY = mybir.AxisListType.XY
    Exp = mybir.ActivationFunctionType.Exp
    Ln = mybir.ActivationFunctionType.Ln
    Ident = mybir.ActivationFunctionType.Identity
    DR = mybir.MatmulPerfMode.DoubleRow

    (gmul,) = _get_custom_ops()
    nc = bacc.Bacc("TRN2", target_bir_lowering=False, debug=False,
                   enable_asserts=False, num_devices=NCORES)

    # DRAM I/O (per core)
    x4b_ap = nc.dram_tensor("x4b", (C, BHW), bf16, kind="ExternalInput").ap()
    x3b_ap = nc.dram_tensor("x3b", (C, BHW), bf16, kind="ExternalInput").ap()
    xq_ap = nc.dram_tensor("xq", (C, 2 * HW), bf16, kind="ExternalInput").ap()
    xv_ap = nc.dram_tensor("xv", (C, 2 * HW), f32r, kind="ExternalInput").ap()
    wqT_ap = nc.dram_tensor("wqT", (C, C), bf16, kind="ExternalInput").ap()
    wkT_ap = nc.dram_tensor("wkT", (C, C), bf16, kind="ExternalInput").ap()
    wvT_ap = nc.dram_tensor("wvT", (C, 128), f32r, kind="ExternalInput").ap()
    bq_ap = nc.dram_tensor("bq", (C, 1), f32, kind="ExternalInput").ap()
    bk_ap = nc.dram_tensor("bk", (C, 1), f32, kind="ExternalInput").ap()
    bvb_ap = nc.dram_tensor("bvb", (128, 1), f32, kind="ExternalInput").ap()
    b128_ap = nc.dram_tensor("b128", (128, 1), f32, kind="ExternalInput").ap()
    out_ap = nc.dram_tensor("out", (512, HW), f32, kind="ExternalOutput").ap()

    SCALE_EFF = (1.0 / 16.0) / 8.0  # /sqrt(C), /8 mean

    with tile.TileContext(nc) as tc:
        with ExitStack() as ctx:
            const = ctx.enter_context(tc.tile_pool(name="const", bufs=1))
            xs = ctx.enter_context(tc.tile_pool(name="xs", bufs=8))
            ps_pool = ctx.enter_context(
                tc.tile_pool(name="ps", bufs=4, space="PSUM"))
            scr = ctx.enter_context(tc.tile_pool(name="scr", bufs=3))
            gp = ctx.enter_context(tc.tile_pool(name="gp", bufs=2))
            fin = ctx.enter_context(tc.tile_pool(name="fin", bufs=2))

            # ---- weights / constants (queue-critical first) ----
            # head DMAs: wk rides sync first (first K conv), wq/xq ride the
            # scalar queue (idle until the first conversions ~20us in),
            # biases on gpsimd ahead of the xt ci1 stream.
            wk_sb = []
            for ci in range(2):
                w = const.tile([128, C], bf16, tag=f"wk{ci}", name=f"wk{ci}")
                nc.sync.dma_start(w[:], wkT_ap[ci * 128:(ci + 1) * 128, :])
                wk_sb.append(w)
            wq_sb, xq_sb = [], []
            for ci in range(2):
                w = const.tile([128, C], bf16, tag=f"wq{ci}", name=f"wq{ci}")
                nc.scalar.dma_start(w[:], wqT_ap[ci * 128:(ci + 1) * 128, :])
                wq_sb.append(w)
                t = const.tile([128, 2 * HW], bf16, tag=f"xq{ci}",
                               name=f"xq{ci}")
                xq_sb.append(t)
            for ci in range(2):
                nc.scalar.dma_start(xq_sb[ci][:, 0:HW],
                                    xq_ap[ci * 128:(ci + 1) * 128, 0:HW])
            bk_sb, bq_sb = [], []
            for ci in range(2):
                b = const.tile([128, 1], f32, tag=f"bk{ci}", name=f"bk{ci}")
                nc.gpsimd.dma_start(b[:], bk_ap[ci * 128:(ci + 1) * 128, :])
                bk_sb.append(b)
            for ci in range(2):
                b = const.tile([128, 1], f32, tag=f"bq{ci}", name=f"bq{ci}")
                nc.gpsimd.dma_start(b[:], bq_ap[ci * 128:(ci + 1) * 128, :])
                bq_sb.append(b)
            bvb_sb = const.tile([128, 1], f32, tag="bvb", name="bvb")
            nc.gpsimd.dma_start(bvb_sb[:], bvb_ap[:, :])
            b128_sb = const.tile([128, 1], f32, tag="b128", name="b128")
            nc.gpsimd.dma_start(b128_sb[:], b128_ap[:, :])

            def emit_head_dmas():
                for ci in range(2):
                    nc.sync.dma_start(
                        xq_sb[ci][:, HW:2 * HW],
                        xq_ap[ci * 128:(ci + 1) * 128, HW:2 * HW])

            ones_row = const.tile([1, 128], f32, tag="ones_row", name="ones_row")
            nc.vector.memset(ones_row[:], 1.0)
            ones_row_bf = const.tile([1, 128], bf16, tag="ones_row_bf",
                                     name="ones_row_bf")
            nc.vector.memset(ones_row_bf[:], 1.0)
            ones_col = const.tile([128, 1], f32, tag="ones_col", name="ones_col")
            nc.vector.memset(ones_col[:], 1.0)
            nbias = const.tile([128, 1], f32, tag="nbias", name="nbias")
            nc.vector.memset(nbias[:], -BETA * CBIAS)
            ident = const.tile([128, 128], f32, tag="ident", name="ident")
            make_identity(nc, ident[:])

            # residents produced on device (fp8 for the DoubleRow score
            # GEMM; 2D tiles viewed as [128, ci_chunk, pix])
            q8 = const.tile([128, 2 * 2 * HW], fp8, tag="q8", name="q8")
            ka8 = const.tile([128, 2 * BHW], fp8, tag="ka8", name="ka8")
            kb8 = const.tile([128, 2 * BHW], fp8, tag="kb8", name="kb8")
            q3 = q8.rearrange("p (s n) -> p s n", s=2)
            ka3 = ka8.rearrange("p (s n) -> p s n", s=2)
            kb3 = kb8.rearrange("p (s n) -> p s n", s=2)
            va_sb = const.tile([128, HW], f32, tag="va", name="va")
            vb_sb = const.tile([128, HW], f32, tag="vb", name="vb")

            # per-(img, q-tile) reductions: col = img*16 + qs.
            # m_all: A-cols hold exp-sums, C-cols exact maxes.
            m_all = const.tile([128, 256], f32, tag="m_all", name="m_all")
            Mka = const.tile([128, 16], f32, tag="Mka", name="Mka")  # (aa, ba)
            Mkb = const.tile([128, 16], f32, tag="Mkb", name="Mkb")  # (ab, bb)

            # ---- Q conv (bf16 -> fp8 out); emitted via emit_head() after
            # the first K convs so the PE starts on K while xq streams ----
            def emit_qconv(n2s=(0, 1)):
              for n2 in n2s:
                for co in range(2):
                    qps = ps_pool.tile([128, 1024], f32, tag="ps", name="qps")
                    for half in range(2):
                        n = n2 * 2 + half
                        for ci in range(2):
                            nc.tensor.matmul(
                                qps[:, half * 512:(half + 1) * 512],
                                wq_sb[ci][:, co * 128:(co + 1) * 128],
                                xq_sb[ci][:, n * 512:(n + 1) * 512],
                                start=(ci == 0), stop=(ci == 1))
                    qdst = q8[:, co * 2048 + n2 * 1024:
                              co * 2048 + (n2 + 1) * 1024]
                    if co == 0:
                        nc.scalar.activation(qdst, qps[:, :], Ident,
                                             bias=bq_sb[co][:])
                    else:
                        nc.vector.tensor_scalar(
                            qdst, qps[:, :], 1.0, bq_sb[co][:],
                            op0=mybir.AluOpType.mult,
                            op1=mybir.AluOpType.add)

            # ---- K conv for one key image (bf16 -> fp8 out) ----
            def prefetch_xt(img):
                src_ap = x4b_ap if img < 8 else x3b_ap
                n2 = img % 8
                xt = []
                for ci in range(2):
                    t = xs.tile([128, 1024], bf16, tag="xt", name="xt")
                    eng = nc.sync if ci == 0 else nc.gpsimd
                    eng.dma_start(
                        t[:], src_ap[ci * 128:(ci + 1) * 128,
                                     n2 * 1024:(n2 + 1) * 1024])
                    xt.append(t)
                return xt

            def emit_kconv(img, xt=None):
                k8 = ka8 if img < 8 else kb8
                n2 = img % 8
                if xt is None:
                    xt = prefetch_xt(img)
                for co in range(2):
                    kps = ps_pool.tile([128, 1024], f32, tag="ps", name="kps")
                    for half in range(2):
                        for ci in range(2):
                            nc.tensor.matmul(
                                kps[:, half * 512:(half + 1) * 512],
                                wk_sb[ci][:, co * 128:(co + 1) * 128],
                                xt[ci][:, half * 512:(half + 1) * 512],
                                start=(ci == 0), stop=(ci == 1))
                    kdst = k8[:, co * BHW + n2 * 1024:
                              co * BHW + (n2 + 1) * 1024]
                    # S:V engine balance: 1 of the 4 conversions per image
                    # pair goes to ScalarE, 3 to VectorE
                    if co == 0 and img % 2 == 0:
                        nc.scalar.activation(kdst, kps[:, :], Ident,
                                             bias=bk_sb[co][:])
                    else:
                        nc.vector.tensor_scalar(
                            kdst, kps[:, :], 1.0, bk_sb[co][:],
                            op0=mybir.AluOpType.mult,
                            op1=mybir.AluOpType.add)

            # ---- V conv (fp32r full-rate, fused weights) ----
            def emit_vconv():
                wv_sb, xv_sb = [], []
                for ci in range(2):
                    w = const.tile([128, 128], f32r, tag=f"wv{ci}", name=f"wv{ci}")
                    nc.gpsimd.dma_start(w[:], wvT_ap[ci * 128:(ci + 1) * 128, :])
                    wv_sb.append(w)
                    t = const.tile([128, 2 * HW], f32r, tag=f"xv{ci}",
                                   name=f"xv{ci}")
                    nc.gpsimd.dma_start(t[:], xv_ap[ci * 128:(ci + 1) * 128, :])
                    xv_sb.append(t)
                for st, v_sb in ((0, va_sb), (1, vb_sb)):
                    vps = ps_pool.tile([128, 1024], f32, tag="ps", name="vps")
                    for half in range(2):
                        for ci in range(2):
                            nc.tensor.matmul(
                                vps[:, half * 512:(half + 1) * 512],
                                wv_sb[ci][:, :],
                                xv_sb[ci][:, st * HW + half * 512:
                                           st * HW + (half + 1) * 512],
                                start=(ci == 0), stop=(ci == 1))
                    nc.vector.tensor_scalar(
                        v_sb[:, :], vps[:, :], 1.0, bvb_sb[:],
                        op0=mybir.AluOpType.mult, op1=mybir.AluOpType.add)

            # ---- scores + consume for one (q-tile, key-image-pair) ----
            def emit_scores(qs, grp):
                ia, ib = grp * 2, grp * 2 + 1
                qcol = qs * 128
                qh = qs // 8

                def score_mms(img):
                    k3 = ka3 if img < 8 else kb3
                    n2 = img % 8
                    t = ps_pool.tile([128, 1024], f32, tag="ps", name="sc_ps")
                    for half in range(2):
                        kcol = n2 * HW + half * 512
                        nc.tensor.matmul(
                            t[:, half * 512:(half + 1) * 512],
                            q3[:, :, qcol:qcol + 128],
                            k3[:, :, kcol:kcol + 512],
                            start=True, stop=True, perf_mode=DR)
                    return t

                def consume(tile_, img):
                    col = img * 16 + qs
                    if _method(img, qh) == 'A':
                        esc = scr.tile([128, 1024], bf16, tag="esc",
                                       name="esc", bufs=3)
                        nc.scalar.activation(
                            esc[:], tile_[:, 0:1024], Exp, bias=nbias[:],
                            scale=BETA, accum_out=m_all[:, col:col + 1])
                    else:
                        nc.vector.reduce_max(
                            m_all[:, col:col + 1], tile_[:, 0:1024], axis=AX)

                t0 = score_mms(ia)
                consume(t0, ia)
                t1 = score_mms(ib)
                consume(t1, ib)

            # ---- per-stream fixup: fold the A-col exp-sums into the sums ----
            # col = img*16 + qs with img = x*4 + y: A-cols for qh=0 are
            # y in {0,1} (grps 0,2 of the stream), for qh=1 y in {2,3};
            # the stream picks x (a: 0..1, b: 2..3).
            # ln(S) is taken via the Schraudolph identity: the fp32 bit
            # pattern of S, read as int32, is ~2^23*(log2(S)+127) (error
            # < 0.086 in log2, i.e. < 0.005 on the /BETA contribution).
            # Summing the BIT PATTERNS of the 4 A-images and scaling by
            # ln2/(2^23*BETA) gives sum ln(S_i)/BETA up to a constant
            # (-4*127*ln2/BETA) that is uniform per gate and cancels in
            # its softmax — no Ln activation needed (the HW Ln table is
            # ~4% inaccurate over this dynamic range).
            i32 = mybir.dt.int32
            # img = x*4 + y2*2 + y1: A-imgs for qh=0 are even (y1=0), for
            # qh=1 odd (y1=1); the stream picks x (a: 0..1, b: 2..3).
            mperm = m_all.rearrange("p (x y2 y1 q) -> p q x y2 y1",
                                    x=4, y2=2, y1=2)

            def emit_stream_fixup(Mdst, lo):
                xb = 0 if lo == 0 else 2
                T1 = gp.tile([128, 16], f32, tag="T1", name="T1")
                T2 = gp.tile([128, 16], f32, tag="T2", name="T2")
                nc.vector.reduce_sum(
                    T2[:, 0:8],
                    mperm[:, 0:8, xb:xb + 2, 0:2, 0].bitcast(i32), axis=AXY)
                nc.vector.reduce_sum(
                    T1[:, 0:8], mperm[:, 0:8, xb:xb + 2, 0:2, 1], axis=AXY)
                nc.vector.reduce_sum(
                    T2[:, 8:16],
                    mperm[:, 8:16, xb:xb + 2, 0:2, 1].bitcast(i32), axis=AXY)
                nc.vector.reduce_sum(
                    T1[:, 8:16], mperm[:, 8:16, xb:xb + 2, 0:2, 0], axis=AXY)
                nc.vector.scalar_tensor_tensor(
                    Mdst[:], T2[:], float(np.log(2.0) / (BETA * 2.0 ** 23)),
                    T1[:],
                    op0=mybir.AluOpType.mult, op1=mybir.AluOpType.add)

            # ---- batched softmax + apply for a pair of gates ----
            def emit_gate_pair(Mpair, specs, tagp):
                E2 = gp.tile([128, 16], f32, tag=f"E2{tagp}", name="E2")
                nc.scalar.activation(E2[:], Mpair[:], Exp, bias=0.0,
                                     scale=SCALE_EFF)
                sr = gp.tile([128, 2], f32, tag=f"sr{tagp}", name="sr")
                nc.vector.reduce_sum(
                    sr[:], E2.rearrange("p (g k) -> p g k", g=2), axis=AX)
                sum_ps = ps_pool.tile([128, 1024], f32, tag="ps", name="sum_ps")
                nc.tensor.matmul(sum_ps[0:2, 0:1], sr[:], ones_col[:],
                                 start=True, stop=True)
                rec2 = gp.tile([2, 1], f32, tag=f"rec{tagp}", name="rec2")
                nc.vector.reciprocal(rec2[:], sum_ps[0:2, 0:1])
                tp = ps_pool.tile([128, 1024], f32, tag="ps", name="tp")
                nc.tensor.transpose(tp[0:1, 0:2], rec2[:], ident[0:2, 0:2])
                recT = gp.tile([1, 2], f32, tag=f"recT{tagp}", name="recT")
                nc.scalar.copy(recT[:], tp[0:1, 0:2])
                bc = ps_pool.tile([128, 1024], f32, tag="ps", name="bc")
                nc.tensor.matmul(bc[:, 0:2], ones_row[:], recT[:],
                                 start=True, stop=True)
                rsb2 = gp.tile([128, 2], f32, tag=f"rsb{tagp}", name="rsb2")
                nc.scalar.copy(rsb2[:], bc[:, 0:2])
                # transpose E (128,16) -> (16,128), flatten to a (1,2048) row
                tpe = ps_pool.tile([128, 1024], f32, tag="ps", name="tpe")
                nc.tensor.transpose(tpe[0:16, 0:128], E2[:], ident[:])
                et = gp.tile([16, 128], bf16, tag=f"et{tagp}", name="et")
                nc.scalar.copy(et[:], tpe[0:16, 0:128])
                grow = gp.tile([1, 2048], bf16, tag=f"grow{tagp}", name="grow")
                nc.sync.dma_start(grow.rearrange("a (t p) -> a t p", t=16),
                                  et[:])
                for gidx, (v_sb, blk) in enumerate(specs):
                    out_t = fin.tile([128, HW], f32, tag="out_t", name="out_t")
                    for half in range(2):
                        # gate row -> 128 partitions on the (idle) Pool
                        # engine, keeping the PE/PSUM out of the gate tail
                        gb = gp.tile([128, 512], bf16, tag=f"gb{tagp}",
                                     name="gb")
                        nc.gpsimd.partition_broadcast(
                            gb[:, :],
                            grow[0:1, gidx * 1024 + half * 512:
                                 gidx * 1024 + (half + 1) * 512])
                        nc.vector._custom_dve(
                            gmul, out=out_t[:, half * 512:(half + 1) * 512],
                            in0=gb[:, :],
                            in1=v_sb[:, half * 512:(half + 1) * 512],
                            s0=b128_sb[:], s1=rsb2[:, gidx:gidx + 1])
                        eng = nc.sync if half == 0 else nc.gpsimd
                        eng.dma_start(
                            out_ap[blk * 128:(blk + 1) * 128,
                                   half * 512:(half + 1) * 512],
                            out_t[:, half * 512:(half + 1) * 512])

            # ---- main schedule ----
            QS_SEQ = [0, 8, 1, 9, 2, 10, 3, 11, 4, 12, 5, 13, 6, 14, 7, 15]
            # K convs prefetched one image pair ahead of their grp so the
            # in-order PE stream never head-of-line blocks on Q-conv
            # readiness during the first grp.
            emit_kconv(0)
            emit_kconv(1)
            emit_head_dmas()
            emit_qconv()
            for grp in range(4):               # x4-stream key images 0..7
                if grp < 3:
                    emit_kconv(2 * grp + 2)
                    emit_kconv(2 * grp + 3)
                if grp == 2:
                    emit_vconv()
                for qs in QS_SEQ:
                    emit_scores(qs, grp)

            emit_stream_fixup(Mka, 0)
            for grp in range(4, 8):            # x3-stream key images 8..15
                emit_kconv(2 * grp)
                emit_kconv(2 * grp + 1)
                for qs in QS_SEQ:
                    emit_scores(qs, grp)
                if grp == 5:
                    # (aa -> block 1, ba -> block 0).  Data-ready since grp 3;
                    # emitted here so its serial softmax chain gets LOW
                    # priority and the scheduler slots its small PE ops where
                    # their inputs are ready (no head-of-line PE stalls).
                    emit_gate_pair(Mka, [(va_sb, 1), (va_sb, 0)], "1")
            emit_stream_fixup(Mkb, 8)
            # (ab -> block 2, bb -> block 3)
            emit_gate_pair(Mkb, [(vb_sb, 2), (vb_sb, 3)], "2")

    nc.compile()
    return nc


def get_nc():
    if "nc" not in _CACHE:
        _CACHE["nc"] = _build_nc()
    return _CACHE["nc"]


def prepare_in_maps(x4, x3, wq, bq, wk, bk, wv, bv, w128, b128):
    bf16 = ml_dtypes.bfloat16
    x4 = np.asarray(x4, np.float32)
    x3 = np.asarray(x3, np.float32)
    X4 = np.ascontiguousarray(x4.transpose(1, 0, 2, 3).reshape(C, BHW))
    X3 = np.ascontiguousarray(x3.transpose(1, 0, 2, 3).reshape(C, BHW))
    X4b = X4.astype(bf16)
    X3b = X3.astype(bf16)
    wq = np.asarray(wq, np.float32)
    wk = np.asarray(wk, np.float32)
    wv = np.asarray(wv, np.float32)
    w128 = np.asarray(w128, np.float32)
    wqT = np.ascontiguousarray(wq.T).astype(bf16)
    wkT = np.ascontiguousarray(wk.T).astype(bf16)
    wvT = np.ascontiguousarray((w128 @ wv).T)          # (256, 128) f32
    bq2 = np.asarray(bq, np.float32).reshape(C, 1)
    bk2 = np.asarray(bk, np.float32).reshape(C, 1)
    bvb = (w128 @ np.asarray(bv, np.float32)).reshape(128, 1).astype(np.float32)
    b128r = np.asarray(b128, np.float32).reshape(128, 1)

    in_maps = []
    for j in range(NCORES):
        sl = slice(j * HW, (j + 1) * HW)
        xq = np.concatenate([X4b[:, sl], X3b[:, sl]], axis=1)
        xv = np.concatenate([X4[:, sl], X3[:, sl]], axis=1)
        in_maps.append({
            "x4b": X4b, "x3b": X3b,
            "xq": np.ascontiguousarray(xq),
            "xv": np.ascontiguousarray(xv),
            "wqT": wqT, "wkT": wkT, "wvT": wvT,
            "bq": bq2, "bk": bk2, "bvb": bvb, "b128": b128r,
        })
    return in_maps


def kernel(**inputs):
    from concourse.bass_utils import run_bass_kernel_spmd

    nc = get_nc()
    in_maps = prepare_in_maps(**inputs)
    res = run_bass_kernel_spmd(nc, in_maps, core_ids=list(range(NCORES)))
    out = np.stack([res.results[c]["out"].reshape(512, 32, 32)
                    for c in range(NCORES)])
    return np.ascontiguousarray(out.astype(np.float32))


# revision 22
# speedup vs baseline: 1.0037x; 1.0037x over previous
"""Trainium2 Bass kernel for nn_AttentionModule_16398185136487.

Math (the reference reduces to this — its trailing softmax is over a size-1
axis, i.e. ones):
  out = concat([x34, a_x4, x43, b_x3], axis=1)            # (8, 512, 32, 32)
  block(qs, ks, v) = gate(qs, ks) * (w128@wv @ x_v + w128@bv) + b128
  gate(qs, ks)[b, hw] = softmax_hw( (1/8) sum_{kb} max_{khw}
                                    (Q_qs[b,hw] . K_ks[kb,khw]) / 16 )

Sharding: core j owns batch image j (its 1024 query pixels for both the x4
and x3 streams) — the per-image softmax is then fully core-local; no
collectives.  The K tensors (all 16 key images) are computed replicated on
every core from the full x4/x3 (a small 1x1 conv vs the score GEMM shard).

Engine plan per core (v2):
  - Q/K convs in bf16 with outputs quantized to fp8e4; the 1024 score
    matmuls per (q-tile, key-image) pair collapse to ONE fp8 DoubleRow
    matmul per 512-key half (K=256 contraction via the 2 stacked ci
    chunks, 2 MACs/cycle) — the score GEMM runs at 2x the bf16 rate and
    with half the LDWEIGHTS.
  - per-image max over 1024 keys is split across the two PSUM-capable
    elementwise engines (GPSIMD cannot touch PSUM; DMA cannot read it):
      method A (ScalarE): one-pass exp(BETA*(s-C)) with accum_out -> the
        per-row exp-sum; log once per A-column later (LSE ~ max for
        BETA=12; the -BETA*C bias keeps exp in fp32 range and its
        constant offset cancels in the per-image softmax because the
        A/C method split is uniform across each gate's 8 q-tiles).
      method C (VectorE): exact reduce_max over the 1024-col 2-bank
        PSUM tile in one op.
    Methods are assigned per (key-image, query-half) in a checkerboard
    so both engines run every iteration; qs order is interleaved
    [0,8,1,9,...] to alternate the halves in time.
  - V path in fp32r (full-rate fp32 matmul) with the two convs fused
    host-side (w128@wv).
  - per-image softmax without max-subtraction (logits are O(1)), gates
    broadcast to 128 partitions via K=1 PE matmuls, and a custom DVE op
    (GMUL_BIAS) applies out = gate_row * V * (1/S) + b128 in one pass.
"""

import numpy as np
import ml_dtypes

B = 8
C = 256
HW = 1024          # 32*32
BHW = B * HW       # 8192
NCORES = 8

BETA = 12.0        # LSE sharpness in raw-score units
CBIAS = 9.0        # exp bias: exp(BETA*(s - CBIAS)) stays in fp32 range

_CACHE = {}


def _ref_gmul_bias(in0, in1, c0, c1, c2):
    return (in0.astype(np.float32) * in1 * c1 + c0).astype(np.float32)


def _get_custom_ops():
    """Register the GMUL_BIAS custom DVE microcode op:
      GMUL_BIAS: out = in0 * in1 * s1 + s0     (s0, s1 per-partition APs)
    """
    if "ops" in _CACHE:
        return _CACHE["ops"]
    import concourse.dve_ops as dve_ops
    from concourse.dve_ops import DveOp
    from concourse.dve_spec import Spec, Src0, Src1, C0, C1, lower
    from concourse.dve_uop import DveOpSpec

    def register(name, spec):
        for op in dve_ops.OPS:
            if op.name == name:
                return op
        shas = {}
        for ver in ("v3", "v4"):
            shas[ver] = DveOpSpec(name=name, opcode=1,
                                  uops=lower(spec, ver=ver),
                                  rd1_en=True).sha(ver)
        op = DveOp(name, spec, subdim=False, uops_sha=shas)
        dve_ops.OPS.append(op)
        dve_ops.CUSTOM_DVE_SPECS[op.name] = op.spec
        dve_ops._SUB_OPCODE_FOR_NAME[op.name] = (
            dve_ops._CUSTOM_DVE_ROW_BASE + len(dve_ops.OPS) - 1)
        assert max(dve_ops._SUB_OPCODE_FOR_NAME.values()) < 0x20
        return op

    gmul = register("GMUL_BIAS",
                    Spec(body=Src0 * Src1 * C1 + C0,
                         reference=_ref_gmul_bias))
    _CACHE["ops"] = (gmul,)
    return _CACHE["ops"]


# method('A'=ScalarE LSE | 'C'=DVE exact max) per (img, qhalf); must be
# uniform across the 8 q-tiles of each gate so the -BETA*CBIAS offset of
# A-images cancels in that gate's softmax.  Image-PARITY assignment makes
# every (q-tile, image-pair) iteration feed BOTH engines (ia->A, ib->C or
# swapped), so ScalarE and VectorE run concurrently instead of in
# alternating bursts.
def _method(img, qh):
    return 'A' if (img + qh) % 2 == 0 else 'C'


def _build_nc():
    from contextlib import ExitStack

    import concourse.bass as bass
    import concourse.mybir as mybir
    import concourse.tile as tile
    from concourse import bacc
    from concourse.masks import make_identity

    f32 = mybir.dt.float32
    f32r = mybir.dt.float32r
    bf16 = mybir.dt.bfloat16
    fp8 = mybir.dt.float8e4
    AX = mybir.AxisListType.X
    AXY = mybir.AxisListType.XY
    Exp = mybir.ActivationFunctionType.Exp
    Ln = mybir.ActivationFunctionType.Ln
    Ident = mybir.ActivationFunctionType.Identity
    DR = mybir.MatmulPerfMode.DoubleRow

    (gmul,) = _get_custom_ops()
    nc = bacc.Bacc("TRN2", target_bir_lowering=False, debug=False,
                   enable_asserts=False, num_devices=NCORES)

    # DRAM I/O (per core)
    x4b_ap = nc.dram_tensor("x4b", (C, BHW), bf16, kind="ExternalInput").ap()
    x3b_ap = nc.dram_tensor("x3b", (C, BHW), bf16, kind="ExternalInput").ap()
    xq_ap = nc.dram_tensor("xq", (C, 2 * HW), fp8, kind="ExternalInput").ap()
    xv_ap = nc.dram_tensor("xv", (C, 2 * HW), f32r, kind="ExternalInput").ap()
    wqT_ap = nc.dram_tensor("wqT", (C, C), fp8, kind="ExternalInput").ap()
    wkT_ap = nc.dram_tensor("wkT", (C, C), bf16, kind="ExternalInput").ap()
    wvT_ap = nc.dram_tensor("wvT", (C, 128), f32r, kind="ExternalInput").ap()
    bq_ap = nc.dram_tensor("bq", (C, 1), f32, kind="ExternalInput").ap()
    bk_ap = nc.dram_tensor("bk", (C, 1), f32, kind="ExternalInput").ap()
    bvb_ap = nc.dram_tensor("bvb", (128, 1), f32, kind="ExternalInput").ap()
    b128_ap = nc.dram_tensor("b128", (128, 1), f32, kind="ExternalInput").ap()
    out_ap = nc.dram_tensor("out", (512, HW), f32, kind="ExternalOutput").ap()

    SCALE_EFF = (1.0 / 16.0) / 8.0  # /sqrt(C), /8 mean

    with tile.TileContext(nc) as tc:
        with ExitStack() as ctx:
            const = ctx.enter_context(tc.tile_pool(name="const", bufs=1))
            xs = ctx.enter_context(tc.tile_pool(name="xs", bufs=8))
            ps_pool = ctx.enter_context(
                tc.tile_pool(name="ps", bufs=4, space="PSUM"))
            scr = ctx.enter_context(tc.tile_pool(name="scr", bufs=3))
            gp = ctx.enter_context(tc.tile_pool(name="gp", bufs=2))
            fin = ctx.enter_context(tc.tile_pool(name="fin", bufs=2))

            # ---- weights / constants (queue-critical first) ----
            # head DMAs: wk rides sync first (first K conv), wq/xq ride the
            # scalar queue (idle until the first conversions ~20us in),
            # biases on gpsimd ahead of the xt ci1 stream.
            wk_sb = []
            for ci in range(2):
                w = const.tile([128, C], bf16, tag=f"wk{ci}", name=f"wk{ci}")
                nc.sync.dma_start(w[:], wkT_ap[ci * 128:(ci + 1) * 128, :])
                wk_sb.append(w)
            wq8 = const.tile([128, 2 * C], fp8, tag="wq8", name="wq8")
            for ci in range(2):
                nc.scalar.dma_start(wq8[:, ci * C:(ci + 1) * C],
                                    wqT_ap[ci * 128:(ci + 1) * 128, :])
            xq8 = const.tile([128, 2 * 2 * HW], fp8, tag="xq8", name="xq8")
            for ci in range(2):
                nc.scalar.dma_start(
                    xq8[:, ci * 2 * HW:ci * 2 * HW + HW],
                    xq_ap[ci * 128:(ci + 1) * 128, 0:HW])
            wq3 = wq8.rearrange("p (s n) -> p s n", s=2)
            xq3 = xq8.rearrange("p (s n) -> p s n", s=2)

            def emit_xq_tail_dmas():
                for ci in range(2):
                    nc.sync.dma_start(
                        xq8[:, ci * 2 * HW + HW:(ci + 1) * 2 * HW],
                        xq_ap[ci * 128:(ci + 1) * 128, HW:2 * HW])
            bk_sb, bq_sb = [], []
            for ci in range(2):
                b = const.tile([128, 1], f32, tag=f"bk{ci}", name=f"bk{ci}")
                nc.gpsimd.dma_start(b[:], bk_ap[ci * 128:(ci + 1) * 128, :])
                bk_sb.append(b)
            for ci in range(2):
                b = const.tile([128, 1], f32, tag=f"bq{ci}", name=f"bq{ci}")
                nc.gpsimd.dma_start(b[:], bq_ap[ci * 128:(ci + 1) * 128, :])
                bq_sb.append(b)
            bvb_sb = const.tile([128, 1], f32, tag="bvb", name="bvb")
            nc.gpsimd.dma_start(bvb_sb[:], bvb_ap[:, :])
            b128_sb = const.tile([128, 1], f32, tag="b128", name="b128")
            nc.gpsimd.dma_start(b128_sb[:], b128_ap[:, :])



            ones_row = const.tile([1, 128], f32, tag="ones_row", name="ones_row")
            nc.vector.memset(ones_row[:], 1.0)
            ones_row_bf = const.tile([1, 128], bf16, tag="ones_row_bf",
                                     name="ones_row_bf")
            nc.vector.memset(ones_row_bf[:], 1.0)
            ones_col = const.tile([128, 1], f32, tag="ones_col", name="ones_col")
            nc.vector.memset(ones_col[:], 1.0)
            nbias = const.tile([128, 1], f32, tag="nbias", name="nbias")
            nc.vector.memset(nbias[:], -BETA * CBIAS)
            ident = const.tile([128, 128], f32, tag="ident", name="ident")
            make_identity(nc, ident[:])

            # residents produced on device (fp8 for the DoubleRow score
            # GEMM; 2D tiles viewed as [128, ci_chunk, pix])
            q8 = const.tile([128, 2 * 2 * HW], fp8, tag="q8", name="q8")
            ka8 = const.tile([128, 2 * BHW], fp8, tag="ka8", name="ka8")
            kb8 = const.tile([128, 2 * BHW], fp8, tag="kb8", name="kb8")
            q3 = q8.rearrange("p (s n) -> p s n", s=2)
            ka3 = ka8.rearrange("p (s n) -> p s n", s=2)
            kb3 = kb8.rearrange("p (s n) -> p s n", s=2)
            va_sb = const.tile([128, HW], f32, tag="va", name="va")
            vb_sb = const.tile([128, HW], f32, tag="vb", name="vb")

            # per-(img, q-tile) reductions: col = img*16 + qs.
            # m_all: A-cols hold exp-sums, C-cols exact maxes.
            m_all = const.tile([128, 256], f32, tag="m_all", name="m_all")
            Mka = const.tile([128, 16], f32, tag="Mka", name="Mka")  # (aa, ba)
            Mkb = const.tile([128, 16], f32, tag="Mkb", name="Mkb")  # (ab, bb)

            # ---- Q conv (bf16 -> fp8 out); emitted via emit_head() after
            # the first K convs so the PE starts on K while xq streams ----
            def emit_qconv(n2s=(0, 1)):
              for n2 in n2s:
                for co in range(2):
                    qps = ps_pool.tile([128, 1024], f32, tag="ps", name="qps")
                    for half in range(2):
                        n = n2 * 2 + half
                        nc.tensor.matmul(
                            qps[:, half * 512:(half + 1) * 512],
                            wq3[:, :, co * 128:(co + 1) * 128],
                            xq3[:, :, n * 512:(n + 1) * 512],
                            start=True, stop=True, perf_mode=DR)
                    qdst = q8[:, co * 2048 + n2 * 1024:
                              co * 2048 + (n2 + 1) * 1024]
                    if co == 0:
                        nc.scalar.activation(qdst, qps[:, :], Ident,
                                             bias=bq_sb[co][:])
                    else:
                        nc.vector.tensor_scalar(
                            qdst, qps[:, :], 1.0, bq_sb[co][:],
                            op0=mybir.AluOpType.mult,
                            op1=mybir.AluOpType.add)

            # ---- K conv for one key image (bf16 -> fp8 out) ----
            def prefetch_xt(img):
                src_ap = x4b_ap if img < 8 else x3b_ap
                n2 = img % 8
                xt = []
                for ci in range(2):
                    t = xs.tile([128, 1024], bf16, tag="xt", name="xt")
                    eng = nc.sync if ci == 0 else nc.gpsimd
                    eng.dma_start(
                        t[:], src_ap[ci * 128:(ci + 1) * 128,
                                     n2 * 1024:(n2 + 1) * 1024])
                    xt.append(t)
                return xt

            def emit_kconv(img, xt=None):
                k8 = ka8 if img < 8 else kb8
                n2 = img % 8
                if xt is None:
                    xt = prefetch_xt(img)
                for co in range(2):
                    kps = ps_pool.tile([128, 1024], f32, tag="ps", name="kps")
                    for half in range(2):
                        for ci in range(2):
                            nc.tensor.matmul(
                                kps[:, half * 512:(half + 1) * 512],
                                wk_sb[ci][:, co * 128:(co + 1) * 128],
                                xt[ci][:, half * 512:(half + 1) * 512],
                                start=(ci == 0), stop=(ci == 1))
                    kdst = k8[:, co * BHW + n2 * 1024:
                              co * BHW + (n2 + 1) * 1024]
                    # S:V engine balance: 1 of the 4 conversions per image
                    # pair goes to ScalarE, 3 to VectorE
                    if co == 0 and img % 2 == 0:
                        nc.scalar.activation(kdst, kps[:, :], Ident,
                                             bias=bk_sb[co][:])
                    else:
                        nc.vector.tensor_scalar(
                            kdst, kps[:, :], 1.0, bk_sb[co][:],
                            op0=mybir.AluOpType.mult,
                            op1=mybir.AluOpType.add)

            # ---- V conv (fp32r full-rate, fused weights) ----
            def emit_vconv():
                wv_sb, xv_sb = [], []
                for ci in range(2):
                    w = const.tile([128, 128], f32r, tag=f"wv{ci}", name=f"wv{ci}")
                    nc.gpsimd.dma_start(w[:], wvT_ap[ci * 128:(ci + 1) * 128, :])
                    wv_sb.append(w)
                    t = const.tile([128, 2 * HW], f32r, tag=f"xv{ci}",
                                   name=f"xv{ci}")
                    nc.gpsimd.dma_start(t[:], xv_ap[ci * 128:(ci + 1) * 128, :])
                    xv_sb.append(t)
                for st, v_sb in ((0, va_sb), (1, vb_sb)):
                    vps = ps_pool.tile([128, 1024], f32, tag="ps", name="vps")
                    for half in range(2):
                        for ci in range(2):
                            nc.tensor.matmul(
                                vps[:, half * 512:(half + 1) * 512],
                                wv_sb[ci][:, :],
                                xv_sb[ci][:, st * HW + half * 512:
                                           st * HW + (half + 1) * 512],
                                start=(ci == 0), stop=(ci == 1))
                    nc.vector.tensor_scalar(
                        v_sb[:, :], vps[:, :], 1.0, bvb_sb[:],
                        op0=mybir.AluOpType.mult, op1=mybir.AluOpType.add)

            # ---- scores + consume for one (q-tile, key-image-pair) ----
            def emit_scores(qs, grp):
                ia, ib = grp * 2, grp * 2 + 1
                qcol = qs * 128
                qh = qs // 8

                def score_mms(img):
                    k3 = ka3 if img < 8 else kb3
                    n2 = img % 8
                    t = ps_pool.tile([128, 1024], f32, tag="ps", name="sc_ps")
                    for half in range(2):
                        kcol = n2 * HW + half * 512
                        nc.tensor.matmul(
                            t[:, half * 512:(half + 1) * 512],
                            q3[:, :, qcol:qcol + 128],
                            k3[:, :, kcol:kcol + 512],
                            start=True, stop=True, perf_mode=DR)
                    return t

                def consume(tile_, img):
                    col = img * 16 + qs
                    if _method(img, qh) == 'A':
                        esc = scr.tile([128, 1024], bf16, tag="esc",
                                       name="esc", bufs=3)
                        nc.scalar.activation(
                            esc[:], tile_[:, 0:1024], Exp, bias=nbias[:],
                            scale=BETA, accum_out=m_all[:, col:col + 1])
                    else:
                        nc.vector.reduce_max(
                            m_all[:, col:col + 1], tile_[:, 0:1024], axis=AX)

                t0 = score_mms(ia)
                consume(t0, ia)
                t1 = score_mms(ib)
                consume(t1, ib)

            # ---- per-stream fixup: fold the A-col exp-sums into the sums ----
            # col = img*16 + qs with img = x*4 + y: A-cols for qh=0 are
            # y in {0,1} (grps 0,2 of the stream), for qh=1 y in {2,3};
            # the stream picks x (a: 0..1, b: 2..3).
            # ln(S) is taken via the Schraudolph identity: the fp32 bit
            # pattern of S, read as int32, is ~2^23*(log2(S)+127) (error
            # < 0.086 in log2, i.e. < 0.005 on the /BETA contribution).
            # Summing the BIT PATTERNS of the 4 A-images and scaling by
            # ln2/(2^23*BETA) gives sum ln(S_i)/BETA up to a constant
            # (-4*127*ln2/BETA) that is uniform per gate and cancels in
            # its softmax — no Ln activation needed (the HW Ln table is
            # ~4% inaccurate over this dynamic range).
            i32 = mybir.dt.int32
            # img = x*4 + y2*2 + y1: A-imgs for qh=0 are even (y1=0), for
            # qh=1 odd (y1=1); the stream picks x (a: 0..1, b: 2..3).
            mperm = m_all.rearrange("p (x y2 y1 q) -> p q x y2 y1",
                                    x=4, y2=2, y1=2)

            def emit_stream_fixup(Mdst, lo):
                xb = 0 if lo == 0 else 2
                T1 = gp.tile([128, 16], f32, tag="T1", name="T1")
                T2 = gp.tile([128, 16], f32, tag="T2", name="T2")
                nc.vector.reduce_sum(
                    T2[:, 0:8],
                    mperm[:, 0:8, xb:xb + 2, 0:2, 0].bitcast(i32), axis=AXY)
                nc.vector.reduce_sum(
                    T1[:, 0:8], mperm[:, 0:8, xb:xb + 2, 0:2, 1], axis=AXY)
                nc.vector.reduce_sum(
                    T2[:, 8:16],
                    mperm[:, 8:16, xb:xb + 2, 0:2, 1].bitcast(i32), axis=AXY)
                nc.vector.reduce_sum(
                    T1[:, 8:16], mperm[:, 8:16, xb:xb + 2, 0:2, 0], axis=AXY)
                nc.vector.scalar_tensor_tensor(
                    Mdst[:], T2[:], float(np.log(2.0) / (BETA * 2.0 ** 23)),
                    T1[:],
                    op0=mybir.AluOpType.mult, op1=mybir.AluOpType.add)

            # ---- batched softmax + apply for a pair of gates ----
            def emit_gate_pair(Mpair, specs, tagp):
                E2 = gp.tile([128, 16], f32, tag=f"E2{tagp}", name="E2")
                nc.scalar.activation(E2[:], Mpair[:], Exp, bias=0.0,
                                     scale=SCALE_EFF)
                sr = gp.tile([128, 2], f32, tag=f"sr{tagp}", name="sr")
                nc.vector.reduce_sum(
                    sr[:], E2.rearrange("p (g k) -> p g k", g=2), axis=AX)
                sum_ps = ps_pool.tile([128, 1024], f32, tag="ps", name="sum_ps")
                nc.tensor.matmul(sum_ps[0:2, 0:1], sr[:], ones_col[:],
                                 start=True, stop=True)
                rec2 = gp.tile([2, 1], f32, tag=f"rec{tagp}", name="rec2")
                nc.vector.reciprocal(rec2[:], sum_ps[0:2, 0:1])
                tp = ps_pool.tile([128, 1024], f32, tag="ps", name="tp")
                nc.tensor.transpose(tp[0:1, 0:2], rec2[:], ident[0:2, 0:2])
                recT = gp.tile([1, 2], f32, tag=f"recT{tagp}", name="recT")
                nc.scalar.copy(recT[:], tp[0:1, 0:2])
                bc = ps_pool.tile([128, 1024], f32, tag="ps", name="bc")
                nc.tensor.matmul(bc[:, 0:2], ones_row[:], recT[:],
                                 start=True, stop=True)
                rsb2 = gp.tile([128, 2], f32, tag=f"rsb{tagp}", name="rsb2")
                nc.scalar.copy(rsb2[:], bc[:, 0:2])
                # transpose E (128,16) -> (16,128), flatten to a (1,2048) row
                tpe = ps_pool.tile([128, 1024], f32, tag="ps", name="tpe")
                nc.tensor.transpose(tpe[0:16, 0:128], E2[:], ident[:])
                et = gp.tile([16, 128], bf16, tag=f"et{tagp}", name="et")
                nc.scalar.copy(et[:], tpe[0:16, 0:128])
                grow = gp.tile([1, 2048], bf16, tag=f"grow{tagp}", name="grow")
                nc.sync.dma_start(grow.rearrange("a (t p) -> a t p", t=16),
                                  et[:])
                # all gate-row broadcasts first (Pool), then the GMULs
                # overlap them instead of ping-ponging
                gbs = []
                for gidx in range(2):
                    for half in range(2):
                        gb = gp.tile([128, 512], bf16, tag=f"gb{tagp}",
                                     name="gb", bufs=4)
                        nc.gpsimd.partition_broadcast(
                            gb[:, :],
                            grow[0:1, gidx * 1024 + half * 512:
                                 gidx * 1024 + (half + 1) * 512])
                        gbs.append(gb)
                for gidx, (v_sb, blk) in enumerate(specs):
                    out_t = fin.tile([128, HW], f32, tag="out_t", name="out_t")
                    for half in range(2):
                        nc.vector._custom_dve(
                            gmul, out=out_t[:, half * 512:(half + 1) * 512],
                            in0=gbs[gidx * 2 + half][:, :],
                            in1=v_sb[:, half * 512:(half + 1) * 512],
                            s0=b128_sb[:], s1=rsb2[:, gidx:gidx + 1])
                        eng = nc.sync if half == 0 else nc.gpsimd
                        eng.dma_start(
                            out_ap[blk * 128:(blk + 1) * 128,
                                   half * 512:(half + 1) * 512],
                            out_t[:, half * 512:(half + 1) * 512])

            # ---- main schedule ----
            QS_SEQ = [0, 8, 1, 9, 2, 10, 3, 11, 4, 12, 5, 13, 6, 14, 7, 15]
            # K convs prefetched one image pair ahead of their grp so the
            # in-order PE stream never head-of-line blocks on Q-conv
            # readiness during the first grp.
            emit_kconv(0)
            emit_kconv(1)
            emit_xq_tail_dmas()
            emit_qconv()
            for grp in range(4):               # x4-stream key images 0..7
                if grp < 3:
                    emit_kconv(2 * grp + 2)
                    emit_kconv(2 * grp + 3)
                if grp == 2:
                    emit_vconv()
                for qs in QS_SEQ:
                    emit_scores(qs, grp)

            emit_stream_fixup(Mka, 0)
            for grp in range(4, 8):            # x3-stream key images 8..15
                emit_kconv(2 * grp)
                emit_kconv(2 * grp + 1)
                for qs in QS_SEQ:
                    emit_scores(qs, grp)
                if grp == 5:
                    # (aa -> block 1, ba -> block 0).  Data-ready since grp 3;
                    # emitted here so its serial softmax chain gets LOW
                    # priority and the scheduler slots its small PE ops where
                    # their inputs are ready (no head-of-line PE stalls).
                    emit_gate_pair(Mka, [(va_sb, 1), (va_sb, 0)], "1")
            emit_stream_fixup(Mkb, 8)
            # (ab -> block 2, bb -> block 3)
            emit_gate_pair(Mkb, [(vb_sb, 2), (vb_sb, 3)], "2")

    nc.compile()
    return nc


def get_nc():
    if "nc" not in _CACHE:
        _CACHE["nc"] = _build_nc()
    return _CACHE["nc"]


def prepare_in_maps(x4, x3, wq, bq, wk, bk, wv, bv, w128, b128):
    bf16 = ml_dtypes.bfloat16
    x4 = np.asarray(x4, np.float32)
    x3 = np.asarray(x3, np.float32)
    X4 = np.ascontiguousarray(x4.transpose(1, 0, 2, 3).reshape(C, BHW))
    X3 = np.ascontiguousarray(x3.transpose(1, 0, 2, 3).reshape(C, BHW))
    X4b = X4.astype(bf16)
    X3b = X3.astype(bf16)
    wq = np.asarray(wq, np.float32)
    wk = np.asarray(wk, np.float32)
    wv = np.asarray(wv, np.float32)
    w128 = np.asarray(w128, np.float32)
    f8 = ml_dtypes.float8_e4m3
    wqT = np.ascontiguousarray(wq.T).astype(f8)
    wkT = np.ascontiguousarray(wk.T).astype(bf16)
    wvT = np.ascontiguousarray((w128 @ wv).T)          # (256, 128) f32
    bq2 = np.asarray(bq, np.float32).reshape(C, 1)
    bk2 = np.asarray(bk, np.float32).reshape(C, 1)
    bvb = (w128 @ np.asarray(bv, np.float32)).reshape(128, 1).astype(np.float32)
    b128r = np.asarray(b128, np.float32).reshape(128, 1)

    in_maps = []
    for j in range(NCORES):
        sl = slice(j * HW, (j + 1) * HW)
        xq = np.concatenate([X4[:, sl], X3[:, sl]], axis=1).astype(f8)
        xv = np.concatenate([X4[:, sl], X3[:, sl]], axis=1)
        in_maps.append({
            "x4b": X4b, "x3b": X3b,
            "xq": np.ascontiguousarray(xq),
            "xv": np.ascontiguousarray(xv),
            "wqT": wqT, "wkT": wkT, "wvT": wvT,
            "bq": bq2, "bk": bk2, "bvb": bvb, "b128": b128r,
        })
    return in_maps


def kernel(**inputs):
    from concourse.bass_utils import run_bass_kernel_spmd

    nc = get_nc()
    in_maps = prepare_in_maps(**inputs)
    res = run_bass_kernel_spmd(nc, in_maps, core_ids=list(range(NCORES)))
    out = np.stack([res.results[c]["out"].reshape(512, 32, 32)
                    for c in range(NCORES)])
    return np.ascontiguousarray(out.astype(np.float32))
